# revision 24
# baseline (speedup 1.0000x reference)
"""Trainium2 Bass kernel for nn_Block_45552423141629 (pre-norm transformer
block with ELU linear attention), SPMD over 8 NeuronCores.

Sharding: sequence dimension N=8192 split into 8 shards of 1024 tokens; the
kv outer-product statistics ([B,H,64,65] incl. ksum) are AllReduce'd across
cores once per batch. Everything else is fully local.

Big GEMMs (q/k/v proj, wo, fc1, fc2) run in fp8e4 DoubleRow mode (2 K-subtiles
per matmul). Weights are pre-scaled by WS=256 host-side so their magnitudes
sit in fp8's normal range; the 1/WS is folded into the PSUM-evacuation
activation scale. Activation transposes are single-instruction [128,1024] DMA
xbar transposes (bf16) followed by a gpsimd bf16->fp8 conversion.

Self-contained: hardcodes shapes from the problem spec.
"""
import contextlib

import numpy as np
import ml_dtypes

import concourse.bass as bass
import concourse.mybir as mybir
import concourse.tile as tile
from concourse import bass_utils
from concourse.vector_clock import ScopedClock

# ---------------------------------------------------------------------------
# Workarounds: this walrus build accepts only ONE sync-wait per instruction.
# Split multi-waits onto unfusable NOPs on the same engine, and do the same
# for the TileContext tail drain.
# ---------------------------------------------------------------------------
_orig_lower = tile.TileContext._lower_ordered_insts


def _split_multi_waits(self, ordered):
    nc = self.nc
    for bb, insts in list(ordered.items()):
        new = []
        changed = False
        for inst in insts:
            si = inst.sync_info
            if si is not None and len(si.on_wait) > 1:
                waits = list(si.on_wait)
                for w in waits[:-1]:
                    nop = mybir.InstNoOp(
                        name=nc.get_next_instruction_name(),
                        ins=[],
                        outs=[],
                        bass_is_fusable=False,
                    )
                    nop.engine = inst.engine
                    nop.sync_info = mybir.SyncInfo(on_wait=[w], on_update=[])
                    new.append(nop)
                inst.sync_info = mybir.SyncInfo(
                    on_wait=[waits[-1]], on_update=list(si.on_update)
                )
                changed = True
            new.append(inst)
        if changed:
            ordered[bb] = new
    return _orig_lower(self, ordered)


if tile.TileContext._lower_ordered_insts is not _split_multi_waits:
    tile.TileContext._lower_ordered_insts = _split_multi_waits


def _patched_drain_and_barrier(self, tick_clock, wait_clock):
    nc = self.nc
    pre = nc.sync.nop(nofuse=True)
    wait_clock.add_sem_waits(pre.ins, ScopedClock({None: tick_clock.global_clock}))
    si = pre.ins.sync_info
    waits = list(si.on_wait) if si is not None else []
    if len(waits) > 1:
        pre.ins.sync_info = mybir.SyncInfo(
            on_wait=[waits[0]], on_update=list(si.on_update)
        )
        for w in waits[1:]:
            n2 = nc.sync.nop(nofuse=True)
            n2.ins.sync_info = mybir.SyncInfo(on_wait=[w], on_update=[])
    nc.sync.drain()
    nc.all_engine_barrier()
    popped = nc._tile_sem_poison_stack.pop()
    assert popped is self._sem_poison
    nc.clear_and_free_semaphores(list(self.sems.allocated().values()))
    nc.all_engine_barrier()


tile.TileContext._drain_and_barrier = _patched_drain_and_barrier

# ---------------------------------------------------------------------------

BF = ml_dtypes.bfloat16
F8 = ml_dtypes.float8_e4m3
F32 = mybir.dt.float32
BF16 = mybir.dt.bfloat16
FP8 = mybir.dt.float8e4
AF = mybir.ActivationFunctionType
ALU = mybir.AluOpType
DR = mybir.MatmulPerfMode.DoubleRow

N_CORES = 8
B, N, D, H, HD, DFF = 4, 8192, 1024, 16, 64, 4096
NLOC = N // N_CORES        # 1024 tokens per core per batch
TC = NLOC // 128           # 8 token chunks per batch
DC = D // 128              # 8 dim chunks
GC = DFF // 128            # 32 ff chunks
NPAIR = H // 2             # 8 head pairs
EPS_LN = 1e-5
EPS_NORM = 1e-6
KVS = 1.0 / 64.0           # kv/ksum fp8 pre-scale (cancels in the ratio)
WS = 256.0                 # fp8 weight pre-scale
RWS = 1.0 / WS

_nc_cache = {}


def _build(has_ckv: bool, has_c2: bool):
    key = (has_ckv, has_c2)
    if key in _nc_cache:
        return _nc_cache[key]

    nc = bass.Bass("TRN2", target_bir_lowering=False, debug=False,
                   num_devices=N_CORES)
    src = nc.dram_tensor("src", [B, NLOC, D], F32, kind="ExternalInput")
    # wq packed [p, m, j, o] = ws*wq[j*128+p, m*128+o] (stationary layout)
    wq = nc.dram_tensor("wq", [128, DC, DC, 128], FP8, kind="ExternalInput")
    # wk/wv/wo packed [p, j, d] = ws*w[j*128+p, d] (moving layout)
    wk = nc.dram_tensor("wk", [128, DC, D], FP8, kind="ExternalInput")
    wv = nc.dram_tensor("wv", [128, DC, D], FP8, kind="ExternalInput")
    wo = nc.dram_tensor("wo", [128, NPAIR, D], FP8, kind="ExternalInput")
    # fc1 packed [mp, p, t, j, o] = fc1[j*128+p, (2mp+t)*128+o] (m-pairs, bf16)
    fc1 = nc.dram_tensor("fc1", [GC // 2, 128, 2, D], BF16, kind="ExternalInput")
    # fc2 packed [p, m, d] = ws*fc2[m*128+p, d]
    fc2 = nc.dram_tensor("fc2", [128, GC, D], FP8, kind="ExternalInput")
    c1 = nc.dram_tensor("c1", [128, GC], F32, kind="ExternalInput")
    cq = nc.dram_tensor("cq", [128, DC], F32, kind="ExternalInput")
    if has_ckv:
        ckv = nc.dram_tensor("ckv", [2, D], F32, kind="ExternalInput")
    if has_c2:
        c2 = nc.dram_tensor("c2", [D], F32, kind="ExternalInput")
    out = nc.dram_tensor("out", [B, NLOC, D], F32, kind="ExternalOutput")

    with tile.TileContext(nc) as tc:
        ctx = contextlib.ExitStack()
        with ctx:
            p_one = ctx.enter_context(tc.tile_pool(name="p_one", bufs=1))
            p_f1 = ctx.enter_context(tc.tile_pool(name="p_f1", bufs=3))
            p_x = ctx.enter_context(tc.tile_pool(name="p_x", bufs=2))
            p_h = ctx.enter_context(tc.tile_pool(name="p_h", bufs=2))
            p_hs = ctx.enter_context(tc.tile_pool(name="p_hs", bufs=2))
            p_hT = ctx.enter_context(tc.tile_pool(name="p_hT", bufs=2))
            p_qT = ctx.enter_context(tc.tile_pool(name="p_qT", bufs=2))
            p_h2T = ctx.enter_context(tc.tile_pool(name="p_h2T", bufs=1))
            p_aT = ctx.enter_context(tc.tile_pool(name="p_aT", bufs=1))
            p_gt = ctx.enter_context(tc.tile_pool(name="p_gt", bufs=1))
            p_k = ctx.enter_context(tc.tile_pool(name="p_k", bufs=2))
            p_v = ctx.enter_context(tc.tile_pool(name="p_v", bufs=2))
            p_s2 = ctx.enter_context(tc.tile_pool(name="p_s2", bufs=2))
            p_ae = ctx.enter_context(tc.tile_pool(name="p_ae", bufs=3))
            p_st = ctx.enter_context(tc.tile_pool(name="p_st", bufs=2))
            p_sm = ctx.enter_context(tc.tile_pool(name="p_sm", bufs=1))
            p_ob = ctx.enter_context(tc.tile_pool(name="p_ob", bufs=2))
            ps_mm = ctx.enter_context(
                tc.tile_pool(name="ps_mm", bufs=6, space="PSUM"))
            ps_kv = ctx.enter_context(
                tc.tile_pool(name="ps_kv", bufs=1, space="PSUM"))
            dram = ctx.enter_context(
                tc.tile_pool(name="dramp", bufs=4, space="DRAM"))
            dram_s = ctx.enter_context(
                tc.tile_pool(name="dramps", bufs=4, space="DRAM"))
            dram_s2 = ctx.enter_context(
                tc.tile_pool(name="drams2", bufs=2 * TC, space="DRAM"))

            # --- constants / resident weights ---
            c1_sb = p_one.tile([128, GC], F32, tag="c1", name="c1")
            nc.sync.dma_start(out=c1_sb, in_=c1.ap())
            cq_sb = p_one.tile([128, DC], F32, tag="cq", name="cq")
            nc.sync.dma_start(out=cq_sb, in_=cq.ap())
            eps_sb = p_one.tile([128, 1], F32, tag="eps", name="eps")
            nc.vector.memset(eps_sb, EPS_LN)
            wq_sb = p_one.tile([128, DC, DC, 128], FP8, tag="wq", name="wq_sb")
            nc.gpsimd.dma_start(out=wq_sb, in_=wq.ap())
            wk_sb = p_one.tile([128, DC, D], FP8, tag="wk", name="wk_sb")
            nc.gpsimd.dma_start(out=wk_sb, in_=wk.ap())
            wv_sb = p_one.tile([128, DC, D], FP8, tag="wv", name="wv_sb")
            nc.gpsimd.dma_start(out=wv_sb, in_=wv.ap())
            wo_sb = p_one.tile([128, NPAIR, D], FP8, tag="wo", name="wo_sb")
            nc.gpsimd.dma_start(out=wo_sb, in_=wo.ap())
            fc2_sb = p_one.tile([128, GC, D], FP8, tag="fc2", name="fc2_sb")
            nc.gpsimd.dma_start(out=fc2_sb, in_=fc2.ap())
            if has_ckv:
                ck_b = p_one.tile([128, D], F32, tag="ckb", name="ckb")
                cv_b = p_one.tile([128, D], F32, tag="cvb", name="cvb")
                ckap = ckv.ap()
                for idx, t in ((0, ck_b), (1, cv_b)):
                    nc.sync.dma_start(
                        out=t,
                        in_=bass.AP(tensor=ckap.tensor, offset=idx * D,
                                    ap=[[0, 128], [1, D]]))
            if has_c2:
                c2_b = p_one.tile([128, D], F32, tag="c2b", name="c2b")
                c2ap = c2.ap()
                nc.sync.dma_start(
                    out=c2_b,
                    in_=bass.AP(tensor=c2ap.tensor, offset=0,
                                ap=[[0, 128], [1, D]]))

            def ln_apply(xt, htile):
                """LN stats on vector, rstd + apply on scalar.

                xt: [128, D] fp32 -> htile [128, D] bf16 (normalized)."""
                st = p_st.tile([128, 2, 6], F32, tag="st", name="st")
                xr = xt.rearrange("p (s f) -> p s f", s=2)
                for s in range(2):
                    nc.vector.bn_stats(out=st[:, s, :], in_=xr[:, s, :])
                mv = p_st.tile([128, 2], F32, tag="mv", name="mv")
                nc.vector.bn_aggr(out=mv, in_=st)
                rstd = p_st.tile([128, 1], F32, tag="rstd", name="rstd")
                nc.scalar.activation(out=rstd, in_=mv[:, 1:2], func=AF.Sqrt,
                                     bias=eps_sb, scale=1.0)
                nc.vector.reciprocal(out=rstd, in_=rstd)
                nmr = p_st.tile([128, 1], F32, tag="nmr", name="nmr")
                nc.vector.scalar_tensor_tensor(
                    out=nmr, in0=mv[:, 0:1], scalar=-1.0, in1=rstd,
                    op0=ALU.mult, op1=ALU.mult)
                nc.scalar.activation(out=htile, in_=xt, func=AF.Identity,
                                     bias=nmr, scale=rstd)

            def phase_A(b):
                # LN1 + transpose + fp8 convert
                hT8 = p_hT.tile([128, DC, NLOC], FP8, tag="hT8", name="hT8")
                for i in range(TC):
                    xt = p_x.tile([128, D], F32, tag="x", name="x")
                    nc.sync.dma_start(
                        out=xt, in_=src.ap()[b, i * 128:(i + 1) * 128, :])
                    h = p_h.tile([128, D], BF16, tag="h", name="h")
                    ln_apply(xt, h)
                    hst = p_hs.tile([128, DC, 128], BF16, tag="hst",
                                    name="hst")
                    nc.sync.dma_start_transpose(hst, h)
                    nc.gpsimd.tensor_copy(
                        out=hT8[:, :, i * 128:(i + 1) * 128], in_=hst)

                return hT8

            def phase_B(b, hT8):
                # k/v projections + incremental kv + AllReduce issue
                pkv = ps_kv.tile([128, NPAIR, 128], F32, tag="kv", name="pkv")
                for i in range(TC):
                    k_t = p_k.tile([128, D], BF16, tag="k", name="k_t")
                    v_t = p_v.tile([128, H, HD + 1], BF16, tag="v", name="v_t")
                    nc.vector.memset(v_t[:, :, HD:HD + 1], 1.0)
                    for ncol in range(2):
                        csl = slice(ncol * 512, (ncol + 1) * 512)
                        # k
                        pk = ps_mm.tile([128, 512], F32, tag="mm", name="pk")
                        for jj in range(DC // 2):
                            nc.tensor.matmul(
                                pk,
                                hT8[:, 2 * jj:2 * jj + 2,
                                    i * 128:(i + 1) * 128],
                                wk_sb[:, 2 * jj:2 * jj + 2, csl],
                                start=(jj == 0), stop=(jj == DC // 2 - 1),
                                perf_mode=DR)
                        if has_ckv:
                            kb = p_ae.tile([128, 512], F32, tag="kb",
                                           name="kb")
                            nc.vector.scalar_tensor_tensor(
                                out=kb, in0=pk, scalar=RWS,
                                in1=ck_b[:, csl], op0=ALU.mult, op1=ALU.add)
                            rl = p_ae.tile([128, 512], BF16, tag="ae",
                                           name="rl")
                            nc.scalar.activation(out=rl, in_=kb, func=AF.Relu)
                            ex = p_ae.tile([128, 512], BF16, tag="ae",
                                           name="ex")
                            nc.scalar.activation(out=ex, in_=kb, func=AF.Exp)
                        else:
                            rl = p_ae.tile([128, 512], BF16, tag="ae",
                                           name="rl")
                            nc.scalar.activation(out=rl, in_=pk, func=AF.Relu,
                                                 scale=RWS)
                            ex = p_ae.tile([128, 512], BF16, tag="ae",
                                           name="ex")
                            nc.scalar.activation(out=ex, in_=pk, func=AF.Exp,
                                                 scale=RWS)
                        nc.vector.scalar_tensor_tensor(
                            out=k_t[:, csl], in0=ex, scalar=1.0, in1=rl,
                            op0=ALU.min, op1=ALU.add)
                        # v
                        pv = ps_mm.tile([128, 512], F32, tag="mm", name="pv")
                        for jj in range(DC // 2):
                            nc.tensor.matmul(
                                pv,
                                hT8[:, 2 * jj:2 * jj + 2,
                                    i * 128:(i + 1) * 128],
                                wv_sb[:, 2 * jj:2 * jj + 2, csl],
                                start=(jj == 0), stop=(jj == DC // 2 - 1),
                                perf_mode=DR)
                        vdst = v_t[:, ncol * 8:(ncol + 1) * 8, 0:HD]
                        pvr = pv.rearrange("p (h e) -> p h e", e=HD)
                        if has_ckv:
                            cvr = cv_b[:, csl].rearrange(
                                "p (h e) -> p h e", e=HD)
                            nc.vector.scalar_tensor_tensor(
                                out=vdst, in0=pvr, scalar=RWS, in1=cvr,
                                op0=ALU.mult, op1=ALU.add)
                        else:
                            nc.vector.tensor_scalar(
                                out=vdst, in0=pvr, scalar1=RWS, scalar2=None,
                                op0=ALU.mult)
                    # accumulate kv for all head pairs from this chunk
                    for hp in range(NPAIR):
                        hA, hB = 2 * hp, 2 * hp + 1
                        nc.tensor.matmul(
                            pkv[0:64, hp, 0:HD + 1],
                            k_t[:, hA * HD:(hA + 1) * HD],
                            v_t[:, hA, :],
                            start=(i == 0), stop=(i == TC - 1),
                            tile_position=(0, 0), skip_group_check=True)
                        nc.tensor.matmul(
                            pkv[64:128, hp, 0:HD + 1],
                            k_t[:, hB * HD:(hB + 1) * HD],
                            v_t[:, hB, :],
                            start=(i == 0), stop=(i == TC - 1),
                            tile_position=(0, 64), skip_group_check=True)

                kv_sb = p_sm.tile([128, NPAIR, HD + 1], F32, tag="kvsb",
                                  name="kvsb")
                nc.vector.tensor_copy(out=kv_sb, in_=pkv[:, :, 0:HD + 1])
                kv_in = dram.tile([128, NPAIR, HD + 1], F32, tag="kvin",
                                  name="kvin")
                kv_out = dram_s.tile([128, NPAIR, HD + 1], F32, tag="kvout",
                                     name="kvout", addr_space="Shared")
                nc.sync.dma_start(out=kv_in, in_=kv_sb)
                nc.gpsimd.collective_compute(
                    "AllReduce", ALU.add,
                    replica_groups=[list(range(N_CORES))],
                    ins=[kv_in.opt()], outs=[kv_out.opt()])

                return kv_out

            def phase_B3(b, hT8):
                # q projection (overlaps AR + next batch A/B)
                qTb = p_qT.tile([128, DC, NLOC], FP8, tag="qTb", name="qTb")
                for m in range(DC):
                    for ncol in range(2):
                        csl = slice(ncol * 512, (ncol + 1) * 512)
                        pq = ps_mm.tile([128, 512], F32, tag="mm", name="pq")
                        for jj in range(DC // 2):
                            nc.tensor.matmul(
                                pq,
                                wq_sb[:, m, 2 * jj:2 * jj + 2, :],
                                hT8[:, 2 * jj:2 * jj + 2, csl],
                                start=(jj == 0), stop=(jj == DC // 2 - 1),
                                perf_mode=DR)
                        rl = p_ae.tile([128, 512], BF16, tag="ae", name="rlq")
                        nc.scalar.activation(out=rl, in_=pq, func=AF.Relu,
                                             bias=cq_sb[:, m:m + 1], scale=RWS)
                        ex = p_ae.tile([128, 512], BF16, tag="ae", name="exq")
                        nc.scalar.activation(out=ex, in_=pq, func=AF.Exp,
                                             bias=cq_sb[:, m:m + 1], scale=RWS)
                        nc.vector.scalar_tensor_tensor(
                            out=qTb[:, m, csl], in0=ex, scalar=1.0, in1=rl,
                            op0=ALU.min, op1=ALU.add)

                return qTb

            def phase_D0(b, kv_out):
                # kv fetch + fp8 prep (no tensor work; issued early so the
                # AllReduce result is staged before the D MMs need it)
                kv_red = p_sm.tile([128, NPAIR, HD + 1], F32, tag="kvred",
                                   name="kvred")
                nc.sync.dma_start(out=kv_red, in_=kv_out)
                kvb = p_sm.tile([128, NPAIR, HD + 1], FP8, tag="kvb",
                                name="kvb")
                # 1/64 pre-scale keeps kv/ksum inside fp8 range; the factor
                # cancels exactly in out/normalizer (eps scaled to match).
                nc.vector.tensor_scalar(
                    out=kvb, in0=kv_red, scalar1=KVS, scalar2=None,
                    op0=ALU.mult)
                ks16s = []
                for hp in range(NPAIR):
                    ks16 = p_sm.tile([128, 16], FP8, tag="ks16", name="ks16",
                                     bufs=NPAIR)
                    nc.vector.memset(ks16, 0.0)
                    nc.vector.tensor_copy(
                        out=ks16[0:64, 2 * hp:2 * hp + 1],
                        in_=kvb[0:64, hp, HD:HD + 1])
                    nc.vector.tensor_copy(
                        out=ks16[64:128, 2 * hp + 1:2 * hp + 2],
                        in_=kvb[64:128, hp, HD:HD + 1])
                    ks16s.append(ks16)
                return kvb, ks16s

            def phase_EGH(b, qTb, d0):
                kvb, ks16s = d0
                # normalizers: accumulate block-diag ksum matmuls
                pn = [ps_mm.tile([16, 512], F32, tag="mm", name="pn")
                      for _ in range(2)]
                for hp in range(NPAIR):
                    for ncol in range(2):
                        nc.tensor.matmul(
                            pn[ncol], ks16s[hp],
                            qTb[:, hp, ncol * 512:(ncol + 1) * 512],
                            start=(hp == 0), stop=(hp == NPAIR - 1),
                            skip_group_check=True)
                rn16 = p_sm.tile([16, NLOC], BF16, tag="rn16", name="rn16")
                for ncol in range(2):
                    nc.vector.tensor_scalar_add(
                        out=pn[ncol], in0=pn[ncol], scalar1=EPS_NORM * KVS)
                    with nc.allow_low_precision(reason="rn broadcast in bf16"):
                        nc.vector.reciprocal(
                            out=rn16[:, ncol * 512:(ncol + 1) * 512],
                            in_=pn[ncol])
                rn_d = dram.tile([16, NLOC], BF16, tag="rnd", name="rnd")
                nc.sync.dma_start(out=rn_d, in_=rn16)

                aT8 = p_aT.tile([128, NPAIR, NLOC], FP8, tag="aT8",
                                name="aT8")
                for hp in range(NPAIR):
                    rnbt = p_ae.tile([128, NLOC], BF16, tag="rnbt",
                                     name="rnbt")
                    rnap = rn_d.opt()
                    for hh in range(2):
                        nc.scalar.dma_start(
                            out=rnbt[hh * 64:(hh + 1) * 64, :],
                            in_=bass.AP(
                                tensor=rnap.tensor,
                                offset=rnap.offset + (2 * hp + hh) * NLOC,
                                ap=[[0, 64], [1, NLOC]]))
                    for ncol in range(2):
                        csl = slice(ncol * 512, (ncol + 1) * 512)
                        rnb = rnbt[:, csl]
                        po = ps_mm.tile([128, 512], F32, tag="mm", name="po")
                        nc.tensor.matmul(
                            po[0:64, :], kvb[0:64, hp, 0:HD],
                            qTb[0:64, hp, csl],
                            start=True, stop=True, tile_position=(0, 0))
                        nc.tensor.matmul(
                            po[64:128, :], kvb[64:128, hp, 0:HD],
                            qTb[64:128, hp, csl],
                            start=True, stop=True, tile_position=(64, 64))
                        nc.vector.tensor_mul(
                            out=aT8[:, hp, csl], in0=po, in1=rnb)

                # ---------------- Phase E: wo + residual + LN2 -----------
                h2Tb = p_h2T.tile([128, DC, NLOC], BF16, tag="h2Tb",
                                  name="h2Tb")
                s2d = [dram_s2.tile([128, D], F32, tag="s2d", name="s2d")
                       for _ in range(TC)]
                for i in range(TC):
                    x2 = p_x.tile([128, D], F32, tag="x", name="x2")
                    nc.sync.dma_start(
                        out=x2, in_=src.ap()[b, i * 128:(i + 1) * 128, :])
                    s2 = p_s2.tile([128, D], F32, tag="s2", name="s2")
                    for ncol in range(2):
                        csl = slice(ncol * 512, (ncol + 1) * 512)
                        py = ps_mm.tile([128, 512], F32, tag="mm", name="py")
                        for hh in range(NPAIR // 2):
                            nc.tensor.matmul(
                                py,
                                aT8[:, 2 * hh:2 * hh + 2,
                                    i * 128:(i + 1) * 128],
                                wo_sb[:, 2 * hh:2 * hh + 2, csl],
                                start=(hh == 0), stop=(hh == NPAIR // 2 - 1),
                                perf_mode=DR)
                        nc.vector.scalar_tensor_tensor(
                            out=s2[:, csl], in0=py, scalar=RWS,
                            in1=x2[:, csl], op0=ALU.mult, op1=ALU.add)
                    nc.scalar.dma_start(out=s2d[i], in_=s2)
                    h2 = p_h.tile([128, D], BF16, tag="h", name="h2")
                    ln_apply(s2, h2)
                    nc.sync.dma_start_transpose(
                        h2Tb[:, :, i * 128:(i + 1) * 128], h2)

                # ---------------- Phase G/H: MLP, per t-half -------------
                gt = p_gt.tile([128, GC, 512], FP8, tag="gt", name="gt")
                for half in range(2):
                    tsl = slice(half * 512, (half + 1) * 512)
                    for mp in range(GC // 2):
                        f1 = p_f1.tile([128, 2, D], BF16, tag="f1", name="f1")
                        nc.gpsimd.dma_start(out=f1, in_=fc1.ap()[mp])
                        for t in range(2):
                            pu = ps_mm.tile([128, 512], F32, tag="mm",
                                            name="pu")
                            f1r = f1[:, t, :].rearrange(
                                "p (j o) -> p j o", o=128)
                            for j in range(DC):
                                nc.tensor.matmul(
                                    pu,
                                    f1r[:, j, :],
                                    h2Tb[:, j, tsl],
                                    start=(j == 0), stop=(j == DC - 1))
                            m = 2 * mp + t
                            nc.scalar.activation(
                                out=gt[:, m, :], in_=pu, func=AF.Gelu,
                                bias=c1_sb[:, m:m + 1], scale=1.0)
                    for ncol in range(2):
                        csl = slice(ncol * 512, (ncol + 1) * 512)
                        for ii in range(4):
                            py2 = ps_mm.tile([128, 512], F32, tag="mm",
                                             name="py2")
                            for mp in range(GC // 2):
                                nc.tensor.matmul(
                                    py2,
                                    gt[:, 2 * mp:2 * mp + 2,
                                       ii * 128:(ii + 1) * 128],
                                    fc2_sb[:, 2 * mp:2 * mp + 2, csl],
                                    start=(mp == 0), stop=(mp == GC // 2 - 1),
                                    perf_mode=DR)
                            i = half * 4 + ii
                            s2c = p_ob.tile([128, 512], F32, tag="s2c",
                                            name="s2c")
                            nc.gpsimd.dma_start(out=s2c, in_=s2d[i][:, csl])
                            ot = p_ob.tile([128, 512], F32, tag="ot",
                                           name="ot")
                            if has_c2:
                                nc.vector.scalar_tensor_tensor(
                                    out=ot, in0=py2, scalar=RWS,
                                    in1=c2_b[:, csl], op0=ALU.mult,
                                    op1=ALU.add)
                                nc.vector.tensor_add(out=ot, in0=ot, in1=s2c)
                            else:
                                nc.vector.scalar_tensor_tensor(
                                    out=ot, in0=py2, scalar=RWS,
                                    in1=s2c, op0=ALU.mult, op1=ALU.add)
                            nc.scalar.dma_start(
                                out=out.ap()[b, i * 128:(i + 1) * 128, csl],
                                in_=ot)

            # ---- pipeline driver: A/B/B3(b+1) all issue before EGH(b) so ----
            # ---- the kv AllReduce(b) and the D-phase serial chain are    ----
            # ---- covered by next-batch projection work on every engine   ----
            hTs = [None] * B
            kvs = [None] * B
            qTs = [None] * B
            d0s = [None] * B
            hTs[0] = phase_A(0)
            kvs[0] = phase_B(0, hTs[0])
            qTs[0] = phase_B3(0, hTs[0])
            for b in range(B):
                if b + 1 < B:
                    hTs[b + 1] = phase_A(b + 1)
                    kvs[b + 1] = phase_B(b + 1, hTs[b + 1])
                d0s[b] = phase_D0(b, kvs[b])
                if b + 1 < B:
                    qTs[b + 1] = phase_B3(b + 1, hTs[b + 1])
                phase_EGH(b, qTs[b], d0s[b])

    _nc_cache[key] = nc
    return nc


def _pack_fp8(a):
    return np.clip(a, -240.0, 240.0).astype(F8)


def prepare_base(inputs):
    """Host-side folds + fp8 packing shared by kernel() and test.py."""
    ln1_w = np.asarray(inputs["ln1_w"], np.float32)
    ln1_b = np.asarray(inputs["ln1_b"], np.float32)
    wq = np.asarray(inputs["wq"], np.float32)
    wk = np.asarray(inputs["wk"], np.float32)
    wv = np.asarray(inputs["wv"], np.float32)
    wo = np.asarray(inputs["wo"], np.float32)
    ln2_w = np.asarray(inputs["ln2_w"], np.float32)
    ln2_b = np.asarray(inputs["ln2_b"], np.float32)
    fc1_w = np.asarray(inputs["fc1_w"], np.float32)
    fc1_b = np.asarray(inputs["fc1_b"], np.float32)
    fc2_w = np.asarray(inputs["fc2_w"], np.float32)
    fc2_b = np.asarray(inputs["fc2_b"], np.float32)

    # [p, m, j, o] = ws * (ln1_w*wq)[j*128+p, m*128+o]
    wqf = _pack_fp8(
        (WS * ln1_w[:, None] * wq).reshape(DC, 128, DC, 128)
        .transpose(1, 2, 0, 3).copy())
    # [p, j, d]
    wkf = _pack_fp8(
        (WS * ln1_w[:, None] * wk).reshape(DC, 128, D).transpose(1, 0, 2)
        .copy())
    wvf = _pack_fp8(
        (WS * ln1_w[:, None] * wv).reshape(DC, 128, D).transpose(1, 0, 2)
        .copy())
    wof = _pack_fp8((WS * wo).reshape(NPAIR, 128, D).transpose(1, 0, 2).copy())
    # [mp, p, t, j*128+o] = (ln2_w*fc1)[j*128+p, (2mp+t)*128+o] in bf16
    f1s = (ln2_w[:, None] * fc1_w).reshape(DC, 128, GC // 2, 2, 128)
    fc1f = f1s.transpose(2, 1, 3, 0, 4).reshape(GC // 2, 128, 2, D).astype(BF).copy()
    fc2f = _pack_fp8(
        (WS * fc2_w).reshape(GC, 128, D).transpose(1, 0, 2).copy())
    cq_v = ln1_b @ wq
    ck_v = ln1_b @ wk
    cv_v = ln1_b @ wv
    c1_v = ln2_b @ fc1_w + fc1_b
    has_ckv = bool(np.any(ck_v) or np.any(cv_v))
    has_c2 = bool(np.any(fc2_b))

    base = {
        "wq": wqf, "wk": wkf, "wv": wvf, "wo": wof,
        "fc1": fc1f, "fc2": fc2f,
        "c1": np.ascontiguousarray(c1_v.reshape(GC, 128).T.astype(np.float32)),
        "cq": np.ascontiguousarray(cq_v.reshape(DC, 128).T.astype(np.float32)),
    }
    if has_ckv:
        base["ckv"] = np.stack([ck_v, cv_v]).astype(np.float32)
    if has_c2:
        base["c2"] = fc2_b.astype(np.float32)
    return base, has_ckv, has_c2


def kernel(**inputs) -> np.ndarray:
    src = np.ascontiguousarray(np.asarray(inputs["src"], dtype=np.float32))
    base, has_ckv, has_c2 = prepare_base(inputs)
    nc = _build(has_ckv, has_c2)
    in_maps = []
    for c in range(N_CORES):
        m = dict(base)
        m["src"] = np.ascontiguousarray(src[:, c * NLOC:(c + 1) * NLOC, :])
        in_maps.append(m)
    res = bass_utils.run_bass_kernel_spmd(
        nc, in_maps, core_ids=list(range(N_CORES)))
    return np.concatenate(
        [res.results[c]["out"] for c in range(N_CORES)], axis=1)



# revision 33
# speedup vs baseline: 1.0759x; 1.0759x over previous
"""Trainium2 Bass kernel for nn_Block_45552423141629 (pre-norm transformer
block with ELU linear attention), SPMD over 8 NeuronCores.

Sharding: sequence dimension N=8192 split into 8 shards of 1024 tokens; the
kv outer-product statistics ([B,H,64,65] incl. ksum) are AllReduce'd across
cores once per batch. Everything else is fully local.

Big GEMMs (q/k/v proj, wo, fc1, fc2) run in fp8e4 DoubleRow mode (2 K-subtiles
per matmul). Weights are pre-scaled by WS=256 host-side so their magnitudes
sit in fp8's normal range; the 1/WS is folded into the PSUM-evacuation
activation scale. Activation transposes are single-instruction [128,1024] DMA
xbar transposes (bf16) followed by a gpsimd bf16->fp8 conversion.

Self-contained: hardcodes shapes from the problem spec.
"""
import contextlib

import numpy as np
import ml_dtypes

import concourse.bass as bass
import concourse.mybir as mybir
import concourse.tile as tile
from concourse import bass_utils
from concourse.vector_clock import ScopedClock

# ---------------------------------------------------------------------------
# Workarounds: this walrus build accepts only ONE sync-wait per instruction.
# Split multi-waits onto unfusable NOPs on the same engine, and do the same
# for the TileContext tail drain.
# ---------------------------------------------------------------------------
_orig_lower = tile.TileContext._lower_ordered_insts


def _split_multi_waits(self, ordered):
    nc = self.nc
    for bb, insts in list(ordered.items()):
        new = []
        changed = False
        for inst in insts:
            si = inst.sync_info
            if si is not None and len(si.on_wait) > 1:
                waits = list(si.on_wait)
                for w in waits[:-1]:
                    nop = mybir.InstNoOp(
                        name=nc.get_next_instruction_name(),
                        ins=[],
                        outs=[],
                        bass_is_fusable=False,
                    )
                    nop.engine = inst.engine
                    nop.sync_info = mybir.SyncInfo(on_wait=[w], on_update=[])
                    new.append(nop)
                inst.sync_info = mybir.SyncInfo(
                    on_wait=[waits[-1]], on_update=list(si.on_update)
                )
                changed = True
            new.append(inst)
        if changed:
            ordered[bb] = new
    return _orig_lower(self, ordered)


if tile.TileContext._lower_ordered_insts is not _split_multi_waits:
    tile.TileContext._lower_ordered_insts = _split_multi_waits


def _patched_drain_and_barrier(self, tick_clock, wait_clock):
    nc = self.nc
    pre = nc.sync.nop(nofuse=True)
    wait_clock.add_sem_waits(pre.ins, ScopedClock({None: tick_clock.global_clock}))
    si = pre.ins.sync_info
    waits = list(si.on_wait) if si is not None else []
    if len(waits) > 1:
        pre.ins.sync_info = mybir.SyncInfo(
            on_wait=[waits[0]], on_update=list(si.on_update)
        )
        for w in waits[1:]:
            n2 = nc.sync.nop(nofuse=True)
            n2.ins.sync_info = mybir.SyncInfo(on_wait=[w], on_update=[])
    nc.sync.drain()
    nc.all_engine_barrier()
    popped = nc._tile_sem_poison_stack.pop()
    assert popped is self._sem_poison
    nc.clear_and_free_semaphores(list(self.sems.allocated().values()))
    nc.all_engine_barrier()


tile.TileContext._drain_and_barrier = _patched_drain_and_barrier

# ---------------------------------------------------------------------------

BF = ml_dtypes.bfloat16
F8 = ml_dtypes.float8_e4m3
F32 = mybir.dt.float32
BF16 = mybir.dt.bfloat16
FP8 = mybir.dt.float8e4
AF = mybir.ActivationFunctionType
ALU = mybir.AluOpType
DR = mybir.MatmulPerfMode.DoubleRow

N_CORES = 8
B, N, D, H, HD, DFF = 4, 8192, 1024, 16, 64, 4096
NLOC = N // N_CORES        # 1024 tokens per core per batch
TC = NLOC // 128           # 8 token chunks per batch
DC = D // 128              # 8 dim chunks
GC = DFF // 128            # 32 ff chunks
NPAIR = H // 2             # 8 head pairs
EPS_LN = 1e-5
EPS_NORM = 1e-6
KVS = 1.0 / 64.0           # kv/ksum fp8 pre-scale (cancels in the ratio)
WS = 256.0                 # fp8 weight pre-scale
RWS = 1.0 / WS

_nc_cache = {}


def _build(has_ckv: bool, has_c2: bool, has_cq: bool = False):
    key = (has_ckv, has_c2, has_cq)
    if key in _nc_cache:
        return _nc_cache[key]

    nc = bass.Bass("TRN2", target_bir_lowering=False, debug=False,
                   num_devices=N_CORES)
    src = nc.dram_tensor("src", [B, NLOC, D], F32, kind="ExternalInput")
    # wq packed [p, m, j, o] = ws*wq[j*128+p, m*128+o] (stationary layout)
    wq = nc.dram_tensor("wq", [128, DC, DC, 128], FP8, kind="ExternalInput")
    # wk/wv/wo packed [p, j, d] = ws*w[j*128+p, d] (moving layout)
    wk = nc.dram_tensor("wk", [128, DC, D], FP8, kind="ExternalInput")
    wv = nc.dram_tensor("wv", [128, DC, D], FP8, kind="ExternalInput")
    wo = nc.dram_tensor("wo", [128, NPAIR, D], FP8, kind="ExternalInput")
    # fc1 packed [mp, p, t, j, o] = fc1[j*128+p, (2mp+t)*128+o] (m-pairs, bf16)
    fc1 = nc.dram_tensor("fc1", [GC // 2, 128, 2, D], BF16, kind="ExternalInput")
    # fc2 packed [p, m, d] = ws*fc2[m*128+p, d]
    fc2 = nc.dram_tensor("fc2", [128, GC, D], FP8, kind="ExternalInput")
    c1 = nc.dram_tensor("c1", [128, GC], F32, kind="ExternalInput")
    cq = nc.dram_tensor("cq", [128, DC], F32, kind="ExternalInput")
    if has_ckv:
        ckv = nc.dram_tensor("ckv", [2, D], F32, kind="ExternalInput")
    if has_c2:
        c2 = nc.dram_tensor("c2", [D], F32, kind="ExternalInput")
    out = nc.dram_tensor("out", [B, NLOC, D], F32, kind="ExternalOutput")

    with tile.TileContext(nc) as tc:
        ctx = contextlib.ExitStack()
        with ctx:
            p_one = ctx.enter_context(tc.tile_pool(name="p_one", bufs=1))
            p_f1 = ctx.enter_context(tc.tile_pool(name="p_f1", bufs=3))
            p_x = ctx.enter_context(tc.tile_pool(name="p_x", bufs=2))
            p_h = ctx.enter_context(tc.tile_pool(name="p_h", bufs=2))
            p_hs = ctx.enter_context(tc.tile_pool(name="p_hs", bufs=2))
            p_hT = ctx.enter_context(tc.tile_pool(name="p_hT", bufs=2))
            p_qT = ctx.enter_context(tc.tile_pool(name="p_qT", bufs=2))
            p_h2T = ctx.enter_context(tc.tile_pool(name="p_h2T", bufs=1))
            p_aT = ctx.enter_context(tc.tile_pool(name="p_aT", bufs=1))
            p_gt = ctx.enter_context(tc.tile_pool(name="p_gt", bufs=1))
            p_k = ctx.enter_context(tc.tile_pool(name="p_k", bufs=2))
            p_v = ctx.enter_context(tc.tile_pool(name="p_v", bufs=2))
            p_s2 = ctx.enter_context(tc.tile_pool(name="p_s2", bufs=2))
            p_ae = ctx.enter_context(tc.tile_pool(name="p_ae", bufs=3))
            p_st = ctx.enter_context(tc.tile_pool(name="p_st", bufs=2))
            p_sm = ctx.enter_context(tc.tile_pool(name="p_sm", bufs=1))
            p_ob = ctx.enter_context(tc.tile_pool(name="p_ob", bufs=2))
            ps_mm = ctx.enter_context(
                tc.tile_pool(name="ps_mm", bufs=6, space="PSUM"))
            ps_kv = ctx.enter_context(
                tc.tile_pool(name="ps_kv", bufs=1, space="PSUM"))
            dram = ctx.enter_context(
                tc.tile_pool(name="dramp", bufs=4, space="DRAM"))
            dram_s = ctx.enter_context(
                tc.tile_pool(name="dramps", bufs=4, space="DRAM"))
            dram_s2 = ctx.enter_context(
                tc.tile_pool(name="drams2", bufs=2 * TC, space="DRAM"))

            # --- constants / resident weights ---
            c1_sb = p_one.tile([128, GC], F32, tag="c1", name="c1")
            nc.sync.dma_start(out=c1_sb, in_=c1.ap())
            cq_sb = p_one.tile([128, DC], F32, tag="cq", name="cq")
            nc.sync.dma_start(out=cq_sb, in_=cq.ap())
            eps_sb = p_one.tile([128, 1], F32, tag="eps", name="eps")
            nc.vector.memset(eps_sb, EPS_LN)
            wq_sb = p_one.tile([128, DC, DC, 128], FP8, tag="wq", name="wq_sb")
            nc.scalar.dma_start(out=wq_sb, in_=wq.ap())
            wk_sb = p_one.tile([128, DC, D], FP8, tag="wk", name="wk_sb")
            nc.scalar.dma_start(out=wk_sb, in_=wk.ap())
            wv_sb = p_one.tile([128, DC, D], FP8, tag="wv", name="wv_sb")
            nc.scalar.dma_start(out=wv_sb, in_=wv.ap())
            wo_sb = p_one.tile([128, NPAIR, D], FP8, tag="wo", name="wo_sb")
            nc.scalar.dma_start(out=wo_sb, in_=wo.ap())
            fc2_sb = p_one.tile([128, GC, D], FP8, tag="fc2", name="fc2_sb")
            nc.scalar.dma_start(out=fc2_sb, in_=fc2.ap())
            if has_ckv:
                ck_b = p_one.tile([128, D], F32, tag="ckb", name="ckb")
                cv_b = p_one.tile([128, D], F32, tag="cvb", name="cvb")
                ckap = ckv.ap()
                for idx, t in ((0, ck_b), (1, cv_b)):
                    nc.sync.dma_start(
                        out=t,
                        in_=bass.AP(tensor=ckap.tensor, offset=idx * D,
                                    ap=[[0, 128], [1, D]]))
            if has_c2:
                c2_b = p_one.tile([128, D], F32, tag="c2b", name="c2b")
                c2ap = c2.ap()
                nc.sync.dma_start(
                    out=c2_b,
                    in_=bass.AP(tensor=c2ap.tensor, offset=0,
                                ap=[[0, 128], [1, D]]))

            def ln_apply(xt, htile):
                """LN stats on vector, rstd + apply on scalar.

                xt: [128, D] fp32 -> htile [128, D] bf16 (normalized)."""
                st = p_st.tile([128, 2, 6], F32, tag="st", name="st")
                xr = xt.rearrange("p (s f) -> p s f", s=2)
                for s in range(2):
                    nc.vector.bn_stats(out=st[:, s, :], in_=xr[:, s, :])
                mv = p_st.tile([128, 2], F32, tag="mv", name="mv")
                nc.vector.bn_aggr(out=mv, in_=st)
                rstd = p_st.tile([128, 1], F32, tag="rstd", name="rstd")
                nc.scalar.activation(out=rstd, in_=mv[:, 1:2], func=AF.Sqrt,
                                     bias=eps_sb, scale=1.0)
                nc.vector.reciprocal(out=rstd, in_=rstd)
                nmr = p_st.tile([128, 1], F32, tag="nmr", name="nmr")
                nc.vector.scalar_tensor_tensor(
                    out=nmr, in0=mv[:, 0:1], scalar=-1.0, in1=rstd,
                    op0=ALU.mult, op1=ALU.mult)
                nc.scalar.activation(out=htile, in_=xt, func=AF.Identity,
                                     bias=nmr, scale=rstd)

            def phase_A(b):
                # LN1 + transpose + fp8 convert
                hT8 = p_hT.tile([128, DC, NLOC], FP8, tag="hT8", name="hT8")
                for i in range(TC):
                    xt = p_x.tile([128, D], F32, tag="x", name="x")
                    nc.sync.dma_start(
                        out=xt, in_=src.ap()[b, i * 128:(i + 1) * 128, :])
                    h = p_h.tile([128, D], BF16, tag="h", name="h")
                    ln_apply(xt, h)
                    hst = p_hs.tile([128, DC, 128], BF16, tag="hst",
                                    name="hst")
                    nc.sync.dma_start_transpose(hst, h)
                    nc.vector.tensor_copy(
                        out=hT8[:, :, i * 128:(i + 1) * 128], in_=hst)

                return hT8

            def phase_B(b, hT8):
                # k/v projections + incremental kv + AllReduce issue
                pkv = ps_kv.tile([128, NPAIR, 128], F32, tag="kv", name="pkv")
                for i in range(TC):
                    k_t = p_k.tile([128, D], BF16, tag="k", name="k_t")
                    v_t = p_v.tile([128, H, HD + 1], BF16, tag="v", name="v_t")
                    nc.vector.memset(v_t[:, :, HD:HD + 1], 1.0)
                    for ncol in range(2):
                        csl = slice(ncol * 512, (ncol + 1) * 512)
                        # k
                        pk = ps_mm.tile([128, 512], F32, tag="mm", name="pk")
                        for jj in range(DC // 2):
                            nc.tensor.matmul(
                                pk,
                                hT8[:, 2 * jj:2 * jj + 2,
                                    i * 128:(i + 1) * 128],
                                wk_sb[:, 2 * jj:2 * jj + 2, csl],
                                start=(jj == 0), stop=(jj == DC // 2 - 1),
                                perf_mode=DR)
                        if has_ckv:
                            kb = p_ae.tile([128, 512], F32, tag="kb",
                                           name="kb")
                            nc.vector.scalar_tensor_tensor(
                                out=kb, in0=pk, scalar=RWS,
                                in1=ck_b[:, csl], op0=ALU.mult, op1=ALU.add)
                            rl = p_ae.tile([128, 512], BF16, tag="ae",
                                           name="rl")
                            nc.scalar.activation(out=rl, in_=kb, func=AF.Relu)
                            ex = p_ae.tile([128, 512], BF16, tag="ae",
                                           name="ex")
                            nc.scalar.activation(out=ex, in_=kb, func=AF.Exp)
                        else:
                            # relu on DVE (keeps the scalar queue EXP-only --
                            # act-table swaps cost 1.3us each)
                            rl = p_ae.tile([128, 512], BF16, tag="ae",
                                           name="rl")
                            nc.vector.tensor_scalar(
                                out=rl, in0=pk, scalar1=0.0, scalar2=RWS,
                                op0=ALU.max, op1=ALU.mult)
                            ex = p_ae.tile([128, 512], BF16, tag="ae",
                                           name="ex")
                            nc.scalar.activation(out=ex, in_=pk, func=AF.Exp,
                                                 scale=RWS)
                        nc.vector.scalar_tensor_tensor(
                            out=k_t[:, csl], in0=ex, scalar=1.0, in1=rl,
                            op0=ALU.min, op1=ALU.add)
                        # v
                        pv = ps_mm.tile([128, 512], F32, tag="mm", name="pv")
                        for jj in range(DC // 2):
                            nc.tensor.matmul(
                                pv,
                                hT8[:, 2 * jj:2 * jj + 2,
                                    i * 128:(i + 1) * 128],
                                wv_sb[:, 2 * jj:2 * jj + 2, csl],
                                start=(jj == 0), stop=(jj == DC // 2 - 1),
                                perf_mode=DR)
                        vdst = v_t[:, ncol * 8:(ncol + 1) * 8, 0:HD]
                        pvr = pv.rearrange("p (h e) -> p h e", e=HD)
                        if has_ckv:
                            cvr = cv_b[:, csl].rearrange(
                                "p (h e) -> p h e", e=HD)
                            nc.vector.scalar_tensor_tensor(
                                out=vdst, in0=pvr, scalar=RWS, in1=cvr,
                                op0=ALU.mult, op1=ALU.add)
                        else:
                            nc.vector.tensor_scalar(
                                out=vdst, in0=pvr, scalar1=RWS, scalar2=None,
                                op0=ALU.mult)
                    # accumulate kv for all head pairs from this chunk
                    for hp in range(NPAIR):
                        hA, hB = 2 * hp, 2 * hp + 1
                        nc.tensor.matmul(
                            pkv[0:64, hp, 0:HD + 1],
                            k_t[:, hA * HD:(hA + 1) * HD],
                            v_t[:, hA, :],
                            start=(i == 0), stop=(i == TC - 1),
                            tile_position=(0, 0), skip_group_check=True)
                        nc.tensor.matmul(
                            pkv[64:128, hp, 0:HD + 1],
                            k_t[:, hB * HD:(hB + 1) * HD],
                            v_t[:, hB, :],
                            start=(i == 0), stop=(i == TC - 1),
                            tile_position=(0, 64), skip_group_check=True)

                kv_sb = p_sm.tile([128, NPAIR, HD + 1], F32, tag="kvsb",
                                  name="kvsb")
                nc.vector.tensor_copy(out=kv_sb, in_=pkv[:, :, 0:HD + 1])
                kv_in = dram.tile([128, NPAIR, HD + 1], F32, tag="kvin",
                                  name="kvin")
                kv_out = dram_s.tile([128, NPAIR, HD + 1], F32, tag="kvout",
                                     name="kvout", addr_space="Shared")
                nc.sync.dma_start(out=kv_in, in_=kv_sb)
                nc.gpsimd.collective_compute(
                    "AllReduce", ALU.add,
                    replica_groups=[list(range(N_CORES))],
                    ins=[kv_in.opt()], outs=[kv_out.opt()])

                return kv_out

            def phase_B3(b, hT8):
                # q projection (overlaps AR + next batch A/B)
                qTb = p_qT.tile([128, DC, NLOC], FP8, tag="qTb", name="qTb")
                for m in range(DC):
                    for ncol in range(2):
                        csl = slice(ncol * 512, (ncol + 1) * 512)
                        pq = ps_mm.tile([128, 512], F32, tag="mm", name="pq")
                        for jj in range(DC // 2):
                            nc.tensor.matmul(
                                pq,
                                wq_sb[:, m, 2 * jj:2 * jj + 2, :],
                                hT8[:, 2 * jj:2 * jj + 2, csl],
                                start=(jj == 0), stop=(jj == DC // 2 - 1),
                                perf_mode=DR)
                        rl = p_ae.tile([128, 512], BF16, tag="ae", name="rlq")
                        if has_cq:
                            nc.scalar.activation(
                                out=rl, in_=pq, func=AF.Relu,
                                bias=cq_sb[:, m:m + 1], scale=RWS)
                        else:
                            nc.vector.tensor_scalar(
                                out=rl, in0=pq, scalar1=0.0, scalar2=RWS,
                                op0=ALU.max, op1=ALU.mult)
                        ex = p_ae.tile([128, 512], BF16, tag="ae", name="exq")
                        nc.scalar.activation(out=ex, in_=pq, func=AF.Exp,
                                             bias=cq_sb[:, m:m + 1], scale=RWS)
                        nc.vector.scalar_tensor_tensor(
                            out=qTb[:, m, csl], in0=ex, scalar=1.0, in1=rl,
                            op0=ALU.min, op1=ALU.add)

                return qTb

            def phase_D0(b, kv_out):
                # kv fetch + fp8 prep (no tensor work; issued early so the
                # AllReduce result is staged before the D MMs need it)
                kv_red = p_sm.tile([128, NPAIR, HD + 1], F32, tag="kvred",
                                   name="kvred")
                # gpsimd-queue DMA: the AR-completion wait must not block the
                # sync queue (per-queue DMA counters make later DMAs on the
                # same queue transitively wait on it); everything behind it on
                # the gpsimd queue (f1/s2c loads) already depends on the AR.
                nc.gpsimd.dma_start(out=kv_red, in_=kv_out)
                kvb = p_sm.tile([128, NPAIR, HD + 1], FP8, tag="kvb",
                                name="kvb")
                # 1/64 pre-scale keeps kv/ksum inside fp8 range; the factor
                # cancels exactly in out/normalizer (eps scaled to match).
                nc.vector.tensor_scalar(
                    out=kvb, in0=kv_red, scalar1=KVS, scalar2=None,
                    op0=ALU.mult)
                ks16s = []
                for hp in range(NPAIR):
                    ks16 = p_sm.tile([128, 16], FP8, tag="ks16", name="ks16",
                                     bufs=NPAIR)
                    nc.vector.memset(ks16, 0.0)
                    nc.vector.tensor_copy(
                        out=ks16[0:64, 2 * hp:2 * hp + 1],
                        in_=kvb[0:64, hp, HD:HD + 1])
                    nc.vector.tensor_copy(
                        out=ks16[64:128, 2 * hp + 1:2 * hp + 2],
                        in_=kvb[64:128, hp, HD:HD + 1])
                    ks16s.append(ks16)
                return kvb, ks16s

            def phase_EGH(b, qTb, d0):
                kvb, ks16s = d0
                # normalizers: accumulate block-diag ksum matmuls
                pn = [ps_mm.tile([16, 512], F32, tag="mm", name="pn")
                      for _ in range(2)]
                for hp in range(NPAIR):
                    for ncol in range(2):
                        nc.tensor.matmul(
                            pn[ncol], ks16s[hp],
                            qTb[:, hp, ncol * 512:(ncol + 1) * 512],
                            start=(hp == 0), stop=(hp == NPAIR - 1),
                            skip_group_check=True)
                rn16 = p_sm.tile([16, NLOC], BF16, tag="rn16", name="rn16")
                for ncol in range(2):
                    nc.vector.tensor_scalar_add(
                        out=pn[ncol], in0=pn[ncol], scalar1=EPS_NORM * KVS)
                    with nc.allow_low_precision(reason="rn broadcast in bf16"):
                        nc.vector.reciprocal(
                            out=rn16[:, ncol * 512:(ncol + 1) * 512],
                            in_=pn[ncol])
                rn_d = dram.tile([16, NLOC], BF16, tag="rnd", name="rnd")
                nc.sync.dma_start(out=rn_d, in_=rn16)

                aT8 = p_aT.tile([128, NPAIR, NLOC], FP8, tag="aT8",
                                name="aT8")
                for hp in range(NPAIR):
                    rnbt = p_ae.tile([128, NLOC], BF16, tag="rnbt",
                                     name="rnbt")
                    rnap = rn_d.opt()
                    for hh in range(2):
                        nc.scalar.dma_start(
                            out=rnbt[hh * 64:(hh + 1) * 64, :],
                            in_=bass.AP(
                                tensor=rnap.tensor,
                                offset=rnap.offset + (2 * hp + hh) * NLOC,
                                ap=[[0, 64], [1, NLOC]]))
                    for ncol in range(2):
                        csl = slice(ncol * 512, (ncol + 1) * 512)
                        rnb = rnbt[:, csl]
                        po = ps_mm.tile([128, 512], F32, tag="mm", name="po")
                        nc.tensor.matmul(
                            po[0:64, :], kvb[0:64, hp, 0:HD],
                            qTb[0:64, hp, csl],
                            start=True, stop=True, tile_position=(0, 0))
                        nc.tensor.matmul(
                            po[64:128, :], kvb[64:128, hp, 0:HD],
                            qTb[64:128, hp, csl],
                            start=True, stop=True, tile_position=(64, 64))
                        nc.vector.tensor_mul(
                            out=aT8[:, hp, csl], in0=po, in1=rnb)

                # ---------------- Phase E: wo + residual + LN2 -----------
                h2Tb = p_h2T.tile([128, DC, NLOC], BF16, tag="h2Tb",
                                  name="h2Tb")
                s2d = [dram_s2.tile([128, D], F32, tag="s2d", name="s2d")
                       for _ in range(TC)]
                for i in range(TC):
                    x2 = p_x.tile([128, D], F32, tag="x", name="x2")
                    nc.sync.dma_start(
                        out=x2, in_=src.ap()[b, i * 128:(i + 1) * 128, :])
                    s2 = p_s2.tile([128, D], F32, tag="s2", name="s2")
                    for ncol in range(2):
                        csl = slice(ncol * 512, (ncol + 1) * 512)
                        py = ps_mm.tile([128, 512], F32, tag="mm", name="py")
                        for hh in range(NPAIR // 2):
                            nc.tensor.matmul(
                                py,
                                aT8[:, 2 * hh:2 * hh + 2,
                                    i * 128:(i + 1) * 128],
                                wo_sb[:, 2 * hh:2 * hh + 2, csl],
                                start=(hh == 0), stop=(hh == NPAIR // 2 - 1),
                                perf_mode=DR)
                        nc.vector.scalar_tensor_tensor(
                            out=s2[:, csl], in0=py, scalar=RWS,
                            in1=x2[:, csl], op0=ALU.mult, op1=ALU.add)
                    nc.scalar.dma_start(out=s2d[i], in_=s2)
                    h2 = p_h.tile([128, D], BF16, tag="h", name="h2")
                    ln_apply(s2, h2)
                    nc.sync.dma_start_transpose(
                        h2Tb[:, :, i * 128:(i + 1) * 128], h2)

                # ---------------- Phase G/H: MLP, per t-half -------------
                gt = p_gt.tile([128, GC, 512], FP8, tag="gt", name="gt")
                for half in range(2):
                    tsl = slice(half * 512, (half + 1) * 512)
                    for mp in range(GC // 2):
                        f1 = p_f1.tile([128, 2, D], BF16, tag="f1", name="f1")
                        nc.gpsimd.dma_start(out=f1, in_=fc1.ap()[mp])
                        for t in range(2):
                            pu = ps_mm.tile([128, 512], F32, tag="mm",
                                            name="pu")
                            f1r = f1[:, t, :].rearrange(
                                "p (j o) -> p j o", o=128)
                            for j in range(DC):
                                nc.tensor.matmul(
                                    pu,
                                    f1r[:, j, :],
                                    h2Tb[:, j, tsl],
                                    start=(j == 0), stop=(j == DC - 1))
                            m = 2 * mp + t
                            nc.scalar.activation(
                                out=gt[:, m, :], in_=pu, func=AF.Gelu,
                                bias=c1_sb[:, m:m + 1], scale=1.0)
                    for ncol in range(2):
                        csl = slice(ncol * 512, (ncol + 1) * 512)
                        for ii in range(4):
                            py2 = ps_mm.tile([128, 512], F32, tag="mm",
                                             name="py2")
                            for mp in range(GC // 2):
                                nc.tensor.matmul(
                                    py2,
                                    gt[:, 2 * mp:2 * mp + 2,
                                       ii * 128:(ii + 1) * 128],
                                    fc2_sb[:, 2 * mp:2 * mp + 2, csl],
                                    start=(mp == 0), stop=(mp == GC // 2 - 1),
                                    perf_mode=DR)
                            i = half * 4 + ii
                            s2c = p_ob.tile([128, 512], F32, tag="s2c",
                                            name="s2c")
                            nc.gpsimd.dma_start(out=s2c, in_=s2d[i][:, csl])
                            ot = p_ob.tile([128, 512], F32, tag="ot",
                                           name="ot")
                            if has_c2:
                                nc.vector.scalar_tensor_tensor(
                                    out=ot, in0=py2, scalar=RWS,
                                    in1=c2_b[:, csl], op0=ALU.mult,
                                    op1=ALU.add)
                                nc.vector.tensor_add(out=ot, in0=ot, in1=s2c)
                            else:
                                nc.vector.scalar_tensor_tensor(
                                    out=ot, in0=py2, scalar=RWS,
                                    in1=s2c, op0=ALU.mult, op1=ALU.add)
                            nc.scalar.dma_start(
                                out=out.ap()[b, i * 128:(i + 1) * 128, csl],
                                in_=ot)

            # ---- pipeline driver: A/B/B3(b+1) all issue before EGH(b) so ----
            # ---- the kv AllReduce(b) and the D-phase serial chain are    ----
            # ---- covered by next-batch projection work on every engine   ----
            hTs = [None] * B
            kvs = [None] * B
            qTs = [None] * B
            d0s = [None] * B
            hTs[0] = phase_A(0)
            kvs[0] = phase_B(0, hTs[0])
            qTs[0] = phase_B3(0, hTs[0])
            for b in range(B):
                if b + 1 < B:
                    hTs[b + 1] = phase_A(b + 1)
                    kvs[b + 1] = phase_B(b + 1, hTs[b + 1])
                d0s[b] = phase_D0(b, kvs[b])
                if b + 1 < B:
                    qTs[b + 1] = phase_B3(b + 1, hTs[b + 1])
                phase_EGH(b, qTs[b], d0s[b])

    _nc_cache[key] = nc
    return nc


def _pack_fp8(a):
    return np.clip(a, -240.0, 240.0).astype(F8)


def prepare_base(inputs):
    """Host-side folds + fp8 packing shared by kernel() and test.py."""
    ln1_w = np.asarray(inputs["ln1_w"], np.float32)
    ln1_b = np.asarray(inputs["ln1_b"], np.float32)
    wq = np.asarray(inputs["wq"], np.float32)
    wk = np.asarray(inputs["wk"], np.float32)
    wv = np.asarray(inputs["wv"], np.float32)
    wo = np.asarray(inputs["wo"], np.float32)
    ln2_w = np.asarray(inputs["ln2_w"], np.float32)
    ln2_b = np.asarray(inputs["ln2_b"], np.float32)
    fc1_w = np.asarray(inputs["fc1_w"], np.float32)
    fc1_b = np.asarray(inputs["fc1_b"], np.float32)
    fc2_w = np.asarray(inputs["fc2_w"], np.float32)
    fc2_b = np.asarray(inputs["fc2_b"], np.float32)

    # [p, m, j, o] = ws * (ln1_w*wq)[j*128+p, m*128+o]
    wqf = _pack_fp8(
        (WS * ln1_w[:, None] * wq).reshape(DC, 128, DC, 128)
        .transpose(1, 2, 0, 3).copy())
    # [p, j, d]
    wkf = _pack_fp8(
        (WS * ln1_w[:, None] * wk).reshape(DC, 128, D).transpose(1, 0, 2)
        .copy())
    wvf = _pack_fp8(
        (WS * ln1_w[:, None] * wv).reshape(DC, 128, D).transpose(1, 0, 2)
        .copy())
    wof = _pack_fp8((WS * wo).reshape(NPAIR, 128, D).transpose(1, 0, 2).copy())
    # [mp, p, t, j*128+o] = (ln2_w*fc1)[j*128+p, (2mp+t)*128+o] in bf16
    f1s = (ln2_w[:, None] * fc1_w).reshape(DC, 128, GC // 2, 2, 128)
    fc1f = f1s.transpose(2, 1, 3, 0, 4).reshape(GC // 2, 128, 2, D).astype(BF).copy()
    fc2f = _pack_fp8(
        (WS * fc2_w).reshape(GC, 128, D).transpose(1, 0, 2).copy())
    cq_v = ln1_b @ wq
    ck_v = ln1_b @ wk
    cv_v = ln1_b @ wv
    c1_v = ln2_b @ fc1_w + fc1_b
    has_ckv = bool(np.any(ck_v) or np.any(cv_v))
    has_c2 = bool(np.any(fc2_b))
    has_cq = bool(np.any(cq_v))

    base = {
        "wq": wqf, "wk": wkf, "wv": wvf, "wo": wof,
        "fc1": fc1f, "fc2": fc2f,
        "c1": np.ascontiguousarray(c1_v.reshape(GC, 128).T.astype(np.float32)),
        "cq": np.ascontiguousarray(cq_v.reshape(DC, 128).T.astype(np.float32)),
    }
    if has_ckv:
        base["ckv"] = np.stack([ck_v, cv_v]).astype(np.float32)
    if has_c2:
        base["c2"] = fc2_b.astype(np.float32)
    return base, (has_ckv, has_c2, has_cq)


def kernel(**inputs) -> np.ndarray:
    src = np.ascontiguousarray(np.asarray(inputs["src"], dtype=np.float32))
    base, flags = prepare_base(inputs)
    nc = _build(*flags)
    in_maps = []
    for c in range(N_CORES):
        m = dict(base)
        m["src"] = np.ascontiguousarray(src[:, c * NLOC:(c + 1) * NLOC, :])
        in_maps.append(m)
    res = bass_utils.run_bass_kernel_spmd(
        nc, in_maps, core_ids=list(range(N_CORES)))
    return np.concatenate(
        [res.results[c]["out"] for c in range(N_CORES)], axis=1)

